# revision 25
# baseline (speedup 1.0000x reference)
"""f32r AttentionBlock, software-pipelined, packed constant tensor.

vs kernel_f32rb:
- GroupNorm for element n+1 is emitted BEFORE attention of element n, so the
  whole GN chain (DVE stats -> PE block-diag agg -> Pool apply) overlaps the
  previous element's attention instead of gating the PE at each element
  boundary.
- x is prefetched two elements ahead (xin bufs=3).
- Weights are DMA'd straight into f32r-typed tiles (no f32 staging pool, no
  on-device rounding passes; the PE rounds f32r operands internally).
- Z row-sums ride the DVE (esum accumulated per exp'd key tile); only the
  128-partition broadcast finish (2 matmuls) stays on the PE.
"""
import numpy as np

B, C, HW = 16, 512, 1024
H = W_SP = 32
G = 16
NCORES = 8
BL = B // NCORES
CT = C // 128
TT = HW // 128
CH = HW // 512
EPS = 1e-5
SC = float(C) ** -0.5


def build_program(nc, reps=1, fast=True):
    import concourse.bass as bass
    import concourse.tile as tile
    from concourse import mybir

    f32 = mybir.dt.float32
    f32r = mybir.dt.float32r
    AF = mybir.ActivationFunctionType
    OP = mybir.AluOpType
    fdt = f32r if fast else f32

    def mm(out, lhsT, rhs, start, stop):
        nc.tensor.matmul(out, lhsT, rhs, start=start, stop=stop)

    x_d = nc.dram_tensor("x", [BL, C, HW], f32, kind="ExternalInput")
    # all constants ride in one packed tensor (fewer dispatch buffers);
    # f32r is bit-identical to f32 in memory, so the weight slices DMA
    # straight into matmul-ready f32r tiles
    wp_d = nc.dram_tensor("wpack", [C, 1668], fdt, kind="ExternalInput")
    y_d = nc.dram_tensor("y", [BL, C, HW], f32, kind="ExternalOutput")

    with tile.TileContext(nc) as tc:
        with (
            tc.tile_pool(name="persist", bufs=1) as persist,
            tc.tile_pool(name="xin", bufs=3) as xin,
            tc.tile_pool(name="hp", bufs=2) as hp,
            tc.tile_pool(name="big", bufs=1) as big,
            tc.tile_pool(name="yout", bufs=3) as yout,
            tc.tile_pool(name="small", bufs=2) as small,
            tc.tile_pool(name="ps_score", bufs=2, space="PSUM") as ps_score,
            tc.tile_pool(name="ps_acc", bufs=4, space="PSUM") as ps_acc,
        ):
            # ---------------- startup ----------------
            x0_t = xin.tile([128, CT, HW], f32, name="x_t")
            for ci in range(CT):
                for s in range(2):
                    nc.sync.dma_start(
                        out=x0_t[:, ci, s * 512:(s + 1) * 512],
                        in_=x_d[0, ci * 128:(ci + 1) * 128, s * 512:(s + 1) * 512],
                    )
            bd_sb = persist.tile([128, 128], f32)
            nc.gpsimd.dma_start(out=bd_sb, in_=wp_d[0:128, 1540:1668].bitcast(f32))
            vecs = persist.tile([128, CT, 4], f32)
            for ci in range(CT):
                nc.gpsimd.dma_start(
                    out=vecs[:, ci, :],
                    in_=wp_d[ci * 128:(ci + 1) * 128, 1536:1540].bitcast(f32),
                )
            W_t = persist.tile([128, CT, C], fdt)
            wvT_t = persist.tile([128, CT, C], fdt)
            woT_t = persist.tile([128, CT, C], fdt)
            for ci in range(CT):
                sl = slice(ci * 128, (ci + 1) * 128)
                nc.gpsimd.dma_start(out=wvT_t[:, ci, :], in_=wp_d[sl, 512:1024])
                nc.sync.dma_start(out=W_t[:, ci, :], in_=wp_d[sl, 0:512])
                nc.sync.dma_start(out=woT_t[:, ci, :], in_=wp_d[sl, 1024:1536])
            eps_sb = persist.tile([128, 1], f32)
            nc.vector.memset(eps_sb, EPS)
            ones_f = persist.tile([128, 128], f32)
            nc.vector.memset(ones_f, 1.0)
            ones_sb = persist.tile([128, 128], fdt)
            nc.vector.tensor_copy(out=ones_sb, in_=ones_f)

            def emit_gn_stats(x_t):
                """DVE-only first half of GroupNorm: per-partition stat pairs
                for all 4 channel tiles, batched into one st2 tile."""
                stats = small.tile([128, CT, 2, 6], f32, name="stats")
                for ci in range(CT):
                    for s in range(2):
                        nc.vector.bn_stats(
                            out=stats[:, ci, s, :],
                            in_=x_t[:, ci, s * 512:(s + 1) * 512],
                        )
                mv = small.tile([128, CT, 2], f32, name="mv")
                for ci in range(CT):
                    nc.vector.bn_aggr(out=mv[:, ci, :], in_=stats[:, ci])
                st2 = small.tile([128, CT, 2], f32, name="st2")
                nc.vector.tensor_copy(out=st2[:, :, 0:1], in_=mv[:, :, 0:1])
                nc.vector.tensor_mul(out=st2[:, :, 1:2], in0=mv[:, :, 0:1], in1=mv[:, :, 0:1])
                nc.vector.tensor_add(out=st2[:, :, 1:2], in0=st2[:, :, 1:2], in1=mv[:, :, 1:2])
                return st2

            def emit_gn_finish(x_t, st2):
                """PE block-diag aggregation + scale/bias + apply; emitted
                after the next element's v matmuls so the PE never waits on
                the DVE stat chain."""
                h_t = hp.tile([128, CT, HW], fdt, name="h_t")
                ps_st = ps_acc.tile([128, CT, 2], f32, tag="acc", name="ps_st")
                nc.tensor.matmul(ps_st, bd_sb, st2, start=True, stop=True)
                mug = small.tile([128, CT, 1], f32, name="mug")
                nc.vector.tensor_copy(out=mug, in_=ps_st[:, :, 0:1])
                tv = small.tile([128, CT, 1], f32, name="tv")
                nc.vector.tensor_mul(out=tv, in0=mug, in1=mug)
                nc.vector.tensor_sub(out=tv, in0=ps_st[:, :, 1:2], in1=tv)
                nc.scalar.activation(out=tv, in_=tv, func=AF.Sqrt, bias=eps_sb, scale=1.0)
                nc.vector.reciprocal(out=tv, in_=tv)
                sc_c = small.tile([128, CT, 1], f32, name="sc_c")
                nc.vector.tensor_mul(out=sc_c, in0=tv, in1=vecs[:, :, 0:1])
                bi_c = small.tile([128, CT, 1], f32, name="bi_c")
                nc.vector.tensor_mul(out=bi_c, in0=mug, in1=sc_c)
                nc.vector.tensor_sub(out=bi_c, in0=vecs[:, :, 1:2], in1=bi_c)
                for ci in range(CT):
                    nc.gpsimd.tensor_scalar(
                        out=h_t[:, ci, :], in0=x_t[:, ci, :],
                        scalar1=sc_c[:, ci, :], scalar2=bi_c[:, ci, :],
                        op0=OP.mult, op1=OP.add,
                    )
                return h_t

            def emit_xwob(x_t):
                # x <- x + (wo bv + bo), the residual-side constant; must run
                # after bn_stats/apply have consumed the raw x
                for ci in range(CT):
                    nc.scalar.activation(
                        out=x_t[:, ci, :], in_=x_t[:, ci, :], func=AF.Identity,
                        bias=vecs[:, ci, 3:4], scale=1.0,
                    )

            def emit_attn_v(h_t):
                v_t = big.tile([128, TT, 512], fdt, name="v_t")
                for tt in range(TT):
                    ps_v = ps_acc.tile([128, 512], f32, tag="acc", name="ps_v")
                    for ci in range(CT):
                        mm(
                            ps_v, h_t[:, ci, tt * 128:(tt + 1) * 128], wvT_t[:, ci, :],
                            start=(ci == 0), stop=(ci == CT - 1),
                        )
                    if tt % 2 == 0:
                        nc.scalar.copy(out=v_t[:, tt, :], in_=ps_v)
                    else:
                        nc.vector.tensor_copy(out=v_t[:, tt, :], in_=ps_v)
                return v_t

            def emit_attn_rest(b, x_t, h_t, v_t):
                u_t = big.tile([128, CT, HW], fdt, name="u_t")
                for cj in range(CT):
                    for ch in range(CH):
                        ps_u = ps_acc.tile([128, 512], f32, tag="acc", name="ps_u")
                        for ci in range(CT):
                            mm(
                                ps_u, W_t[:, ci, cj * 128:(cj + 1) * 128],
                                h_t[:, ci, ch * 512:(ch + 1) * 512],
                                start=(ci == 0), stop=(ci == CT - 1),
                            )
                        if (cj + ch) % 2 == 0:
                            nc.vector.tensor_scalar_add(
                                out=u_t[:, cj, ch * 512:(ch + 1) * 512], in0=ps_u,
                                scalar1=vecs[:, cj, 2:3],
                            )
                        else:
                            nc.scalar.activation(
                                out=u_t[:, cj, ch * 512:(ch + 1) * 512], in_=ps_u,
                                func=AF.Identity, bias=vecs[:, cj, 2:3], scale=1.0,
                            )

                eT_t = big.tile([128, TT, HW], fdt, name="eT_t")
                esum = big.tile([128, HW], fdt, name="esum")
                for jt in range(TT):
                    ps_s = ps_score.tile([128, CH, 512], f32, name="ps_s")
                    for ch in range(CH):
                        for cj in range(CT):
                            mm(
                                ps_s[:, ch, :], h_t[:, cj, jt * 128:(jt + 1) * 128],
                                u_t[:, cj, ch * 512:(ch + 1) * 512],
                                start=(cj == 0), stop=(cj == CT - 1),
                            )
                    for ch in range(CH):
                        nc.scalar.activation(
                            out=eT_t[:, jt, ch * 512:(ch + 1) * 512], in_=ps_s[:, ch, :],
                            func=AF.Exp, scale=SC,
                        )
                    if jt == 1:
                        nc.vector.tensor_add(
                            out=esum, in0=eT_t[:, 0, :], in1=eT_t[:, 1, :]
                        )
                    elif jt > 1:
                        nc.vector.tensor_add(out=esum, in0=esum, in1=eT_t[:, jt, :])

                # oT c=0 matmul groups go first so the PE streams while the
                # esum chain (DVE) drains; the Z finish lands in their shadow
                oT_t = big.tile([128, CT, HW], fdt, name="oT_t")
                ps_o0 = []
                for ch in range(CH):
                    ps_o = ps_acc.tile([128, 512], f32, tag="acc", name="ps_o")
                    for jt in range(TT):
                        mm(
                            ps_o, v_t[:, jt, 0:128],
                            eT_t[:, jt, ch * 512:(ch + 1) * 512],
                            start=(jt == 0), stop=(jt == TT - 1),
                        )
                    ps_o0.append(ps_o)

                invZ_t = big.tile([128, HW], f32, name="invZ_t")
                for ch in range(CH):
                    ps_z = ps_acc.tile([128, 512], f32, tag="acc", name="ps_z")
                    mm(
                        ps_z, ones_sb, esum[:, ch * 512:(ch + 1) * 512],
                        start=True, stop=True,
                    )
                    nc.vector.reciprocal(out=invZ_t[:, ch * 512:(ch + 1) * 512], in_=ps_z)
                for ch in range(CH):
                    sl = slice(ch * 512, (ch + 1) * 512)
                    nc.vector.tensor_mul(
                        out=oT_t[:, 0, sl], in0=ps_o0[ch], in1=invZ_t[:, sl]
                    )

                for c in range(1, CT):
                    for ch in range(CH):
                        ps_o = ps_acc.tile([128, 512], f32, tag="acc", name="ps_o")
                        for jt in range(TT):
                            mm(
                                ps_o, v_t[:, jt, c * 128:(c + 1) * 128],
                                eT_t[:, jt, ch * 512:(ch + 1) * 512],
                                start=(jt == 0), stop=(jt == TT - 1),
                            )
                        sl = slice(ch * 512, (ch + 1) * 512)
                        nc.vector.tensor_mul(
                            out=oT_t[:, c, sl], in0=ps_o, in1=invZ_t[:, sl]
                        )

                for cp in range(CT):
                    y_t = yout.tile([128, HW], f32, name="y_t")
                    for ch in range(CH):
                        ps_f = ps_acc.tile([128, 512], f32, tag="acc", name="ps_f")
                        for c in range(CT):
                            mm(
                                ps_f, woT_t[:, c, cp * 128:(cp + 1) * 128],
                                oT_t[:, c, ch * 512:(ch + 1) * 512],
                                start=(c == 0), stop=(c == CT - 1),
                            )
                        sl = slice(ch * 512, (ch + 1) * 512)
                        nc.vector.tensor_add(
                            out=y_t[:, sl], in0=ps_f, in1=x_t[:, cp, sl]
                        )
                        nc.sync.dma_start(
                            out=y_d[b, cp * 128:(cp + 1) * 128, sl], in_=y_t[:, sl]
                        )

            # ---------------- software-pipelined element loop ----------------
            border = [b for _ in range(reps) for b in range(BL)]
            xts = {0: x0_t}
            if len(border) > 1:
                x1_t = xin.tile([128, CT, HW], f32, name="x_t")
                for ci in range(CT):
                    for s in range(2):
                        nc.sync.dma_start(
                            out=x1_t[:, ci, s * 512:(s + 1) * 512],
                            in_=x_d[border[1], ci * 128:(ci + 1) * 128, s * 512:(s + 1) * 512],
                        )
                xts[1] = x1_t
            hts = {0: emit_gn_finish(xts[0], emit_gn_stats(xts[0]))}
            emit_xwob(xts[0])
            for bi, b in enumerate(border):
                if bi + 2 < len(border):
                    nxt = xin.tile([128, CT, HW], f32, name="x_t")
                    for ci in range(CT):
                        for s in range(2):
                            nc.sync.dma_start(
                                out=nxt[:, ci, s * 512:(s + 1) * 512],
                                in_=x_d[border[bi + 2], ci * 128:(ci + 1) * 128, s * 512:(s + 1) * 512],
                            )
                    xts[bi + 2] = nxt
                # next element's DVE stat pass is emitted before this
                # element's attention; its PE/Pool finish lands after the
                # v matmuls so the PE never idles on the stat chain
                st2_next = emit_gn_stats(xts[bi + 1]) if bi + 1 < len(border) else None
                h_t = hts.pop(bi)
                v_t = emit_attn_v(h_t)
                if st2_next is not None:
                    hts[bi + 1] = emit_gn_finish(xts[bi + 1], st2_next)
                emit_attn_rest(b, xts.pop(bi), h_t, v_t)
                if bi + 1 < len(border):
                    emit_xwob(xts[bi + 1])
    return nc


def _const_inputs():
    bd = np.zeros((128, 128), np.float32)
    for g in range(128 // G):
        bd[g * G:(g + 1) * G, g * G:(g + 1) * G] = 1.0 / G
    return {"bd16": bd}


def prep_inputs(inputs):
    x = np.ascontiguousarray(np.asarray(inputs["x"], dtype=np.float32)).reshape(B, C, HW)
    wq = np.asarray(inputs["wq"], dtype=np.float32)
    wk = np.asarray(inputs["wk"], dtype=np.float32)
    wv = np.asarray(inputs["wv"], dtype=np.float32)
    wo = np.asarray(inputs["wo"], dtype=np.float32)
    bq = np.asarray(inputs["bq"], dtype=np.float32).reshape(C)
    bv = np.asarray(inputs["bv"], dtype=np.float32).reshape(C)
    bo = np.asarray(inputs["bo"], dtype=np.float32).reshape(C)
    nw = np.asarray(inputs["norm_w"], dtype=np.float32).reshape(C)
    nb = np.asarray(inputs["norm_b"], dtype=np.float32).reshape(C)
    gk = wk.T @ bq
    wob = wo @ bv + bo
    vecs = np.stack([nw, nb, gk, wob], axis=1)
    bd = _const_inputs()["bd16"]
    wpack = np.zeros((C, 1668), np.float32)
    wpack[:, 0:512] = wq.T @ wk
    wpack[:, 512:1024] = wv.T
    wpack[:, 1024:1536] = wo.T
    wpack[:, 1536:1540] = vecs
    wpack[0:128, 1540:1668] = bd
    base = {"wpack": np.ascontiguousarray(wpack)}
    return base, x


def run_hw(inputs, trace=False):
    from concourse import bacc
    from concourse.bass_utils import run_bass_kernel_spmd

    base, x = prep_inputs(inputs)

    nc = bacc.Bacc("TRN2", target_bir_lowering=False)
    build_program(nc)
    nc.finalize()

    in_maps = [
        {**base, "x": np.ascontiguousarray(x[i * BL:(i + 1) * BL])}
        for i in range(NCORES)
    ]
    try:
        res = run_bass_kernel_spmd(nc, in_maps, list(range(NCORES)), trace=trace)
    except Exception:
        res = run_bass_kernel_spmd(nc, in_maps, list(range(NCORES)), trace=trace)
    y = np.concatenate([res.results[i]["y"] for i in range(NCORES)], axis=0)
    return y.reshape(B, C, H, W_SP).astype(np.float32), res


def kernel(**inputs):
    y, _ = run_hw(inputs, trace=False)
    return y
